# revision 36
# baseline (speedup 1.0000x reference)
"""Multi-head causal attention (B=2, S=2048, D=2048, H=16, HD=128) on 8 TRN2
NeuronCores.

Sharding: data-parallel over batch (2 groups of 4 cores) x tensor-parallel
over heads (4 heads per core).  Each core computes q/k/v projections for its
512 columns (4 heads), causal attention for those heads, and a partial
(contraction-sharded) wo product.  The 4 partial outputs per batch are summed
on the host (the "all-reduce after wo" of the sharding hint).

Everything on-chip is computed in transposed orientation:
  xT [d, s] (host pre-transposed), qT/kT [j, s], scores^T [t, s], out^T [j2, s]
so every matmul contraction lands on the partition axis with zero on-chip
transposes.

v2 performance structure (vs the 331us v1):
  * fine-grained weave: attention group units (whose pace is set by the exp
    activation engine) are interleaved unit-by-unit with projection matmul
    units (pure PE feeders), so the PE never waits on exp.
  * softmax denominators moved off the PE: per-group column sums run as
    gpsimd partition reductions (engine ~9% busy) instead of ones-matmuls.
  * finer causal staircase: diagonal key tiles computed at widths
    512/384/128(packed)/256 instead of 512/512/256/256 (-256 PE rows and
    -256 exp columns per head-chunk); masks shrink to [128,128] multiplies.
  * all PSUM banks placed by hand via one pool with 8 tags: projections use
    2 banks (chunk-0: 4), scores rotate over 5, pv accumulator holds 1.
  * output stores ride the sync DMA queue only - the scalar engine runs
    exps exclusively after the weight prologue.
  * next chunk's x tiles prefetch on the sync queue a full phase early.

Softmax uses exp without max-subtraction (scores are O(4), exact in fp32)
with causal masking by a precomputed 0/1 lower-triangle mask applied
post-exp (exact zeros, matching the reference's exp(-1e9) == 0 underflow).
"""

import ml_dtypes
import numpy as np

import concourse.bass as bass
import concourse.tile as tile
from concourse import bacc, bass_isa, mybir
from concourse.bass_utils import run_bass_kernel_spmd

B, S, D = 2, 2048, 2048
H, HD = 16, 128
P = 128
JL = 512          # local q/k/v columns per core (4 heads)
NH = 4            # heads per core
CHUNK = 512       # s-chunk
NCH = S // CHUNK  # 4
DT = D // P       # 16 d-tiles
NT = S // P       # 16 t-tiles
SCALE = 1.0 / float(np.sqrt(HD))

F32 = mybir.dt.float32
BF16 = mybir.dt.bfloat16


def build_kernel():
    nc = bacc.Bacc("TRN2", target_bir_lowering=False, debug=False, num_devices=8)
    xT = nc.dram_tensor("xT", [D, S], BF16, kind="ExternalInput").ap()
    wqT = nc.dram_tensor("wqT", [D, JL], BF16, kind="ExternalInput").ap()
    wkT = nc.dram_tensor("wkT", [D, JL], BF16, kind="ExternalInput").ap()
    wvT = nc.dram_tensor("wvT", [D, JL], BF16, kind="ExternalInput").ap()
    woT = nc.dram_tensor("woT", [JL, D], BF16, kind="ExternalInput").ap()
    outT = nc.dram_tensor("outT", [D, S], BF16, kind="ExternalOutput").ap()

    with tile.TileContext(nc) as tc:
        with (
            tc.tile_pool(name="persist", bufs=1) as persist,
            tc.tile_pool(name="xt", bufs=2) as xt_pool,
            tc.tile_pool(name="qt", bufs=3) as qt_pool,
            tc.tile_pool(name="exp", bufs=12) as exp_pool,
            tc.tile_pool(name="gs", bufs=2) as gs_pool,
            tc.tile_pool(name="ot", bufs=10) as ot_pool,
            tc.tile_pool(name="rb", bufs=2) as rb_pool,
            tc.tile_pool(name="osb", bufs=6) as osb_pool,
            tc.tile_pool(name="ps", bufs=1, space="PSUM") as ps,
        ):
            # --- 0/1 lower-triangle mask: master[p, u] = 1 iff u >= p ---
            master_f = persist.tile([P, P], F32, name="master_f")
            nc.gpsimd.memset(master_f[:], 1.0)
            nc.gpsimd.affine_select(
                out=master_f[:], in_=master_f[:], pattern=[[1, P]],
                compare_op=mybir.AluOpType.is_ge, fill=0.0,
                base=0, channel_multiplier=-1,
            )
            master = persist.tile([P, P], BF16, name="master")
            nc.vector.tensor_copy(master[:], master_f[:])
            # all-ones [128,128] stationary: the denominator matmul then
            # lands the column sum on EVERY psum partition - the partition
            # broadcast comes free with the reduce
            ones_f = persist.tile([P, P], F32, name="ones_f")
            nc.vector.memset(ones_f[:], 1.0)
            ones = persist.tile([P, P], BF16, name="ones")
            nc.vector.tensor_copy(ones[:], ones_f[:])

            # --- resident weights + chunk-0 x: (wk_d, x_d) pairs land first,
            # alternating queues so pair d is complete after ~d/2 slots; then
            # wq, then wv, then woT. ---
            wk_t = [persist.tile([P, JL], BF16, name=f"wk{d}") for d in range(DT)]
            wq_t = [persist.tile([P, JL], BF16, name=f"wq{d}") for d in range(DT)]
            wv_t = [persist.tile([P, JL], BF16, name=f"wv{d}") for d in range(DT)]
            woT_t = [persist.tile([P, D], BF16, name=f"woT{h}") for h in range(NH)]

            xt0 = [xt_pool.tile([P, CHUNK], BF16, name=f"xt{d}", tag=f"xt{d}")
                   for d in range(DT)]

            # kT per (head, chunk): no writer/reader tile overlap across the
            # software pipeline.  v per key tile as before.
            kT_t = [[persist.tile([P, CHUNK], BF16, name=f"kT{h}_{c}")
                     for c in range(NCH)] for h in range(NH)]
            v_t = [persist.tile([P, JL], BF16, name=f"v{t}") for t in range(NT)]

            xt_of = {0: xt0}  # chunk -> xt tiles
            qt_of = {}        # chunk -> qt tiles
            ots_of = {}       # chunk -> normalized per-head attention outputs

            SCORE_BANKS = ["b2", "b3", "b4", "b5", "b6"]
            score_rr = [0]

            def next_score_bank():
                t = SCORE_BANKS[score_rr[0] % len(SCORE_BANKS)]
                score_rr[0] += 1
                return t

            # ---------------- projections ----------------
            def xpre_units(c):
                """Prefetch chunk c's x tiles on the sync queue (4 DMAs per
                unit, no PE cost)."""
                ssl = slice(c * CHUNK, (c + 1) * CHUNK)
                xt = [xt_pool.tile([P, CHUNK], BF16, name=f"xt{d}", tag=f"xt{d}")
                      for d in range(DT)]
                xt_of[c] = xt

                def issue(d0):
                    for d in range(d0, d0 + 4):
                        nc.sync.dma_start(out=xt[d][:],
                                          in_=xT[d * P:(d + 1) * P, ssl])
                return [lambda d0=d0: issue(d0) for d0 in range(0, DT, 4)]

            def proj0_units():
                """Chunk-0 projections, j-parallel over 4 banks (DMA-paced
                start); one unit = 4 matmuls for one d.  The (x_d, wk_d) DMA
                pair is issued inside unit d so each unit's semaphore wait
                covers only its own pair (a bulk prologue coalesces all 16
                DMAs onto one counter and the first matmul then waits ~4us
                for the full set).  Returns (k_units, qv_units)."""
                kunits = []
                units = []
                state = {}
                xt = xt_of[0]

                def start(kind, base=0):
                    state[kind] = [ps.tile([P, CHUNK], F32, name=f"p{kind}{j}",
                                           tag=f"b{base + j}") for j in range(4)]

                def kqstep(d):
                    # stream [x_d, wk_d, wq_d] and run BOTH the k and q
                    # matmuls for d: consumption (~1.7us/d of PE) matches the
                    # two queues' supply (~1.65us/d), so the DMA-paced head
                    # runs with the PE ~fully fed.  k accumulates on b0-b3,
                    # q on b4-b7 (scores/oa start only in loop 1).
                    xe = nc.sync if d % 2 == 0 else nc.scalar
                    we = nc.scalar if d % 2 == 0 else nc.sync
                    if d == 0:
                        # fine-grained first transfers: the opening matmul
                        # gates on wk0's j0 slice (32KB) + x0's first half
                        # (64KB) instead of two full 131KB tiles.
                        HC = CHUNK // 2
                        xe.dma_start(out=xt[0][:, 0:HC], in_=xT[0:P, 0:HC])
                        for j in range(4):
                            we.dma_start(
                                out=wk_t[0][:, j * P:(j + 1) * P],
                                in_=wkT[0:P, j * P:(j + 1) * P])
                        xe.dma_start(out=xt[0][:, HC:CHUNK],
                                     in_=xT[0:P, HC:CHUNK])
                        we.dma_start(out=wq_t[0][:], in_=wqT[0:P, :])
                        for j in range(4):
                            nc.tensor.matmul(
                                state["k"][j][:, 0:HC],
                                wk_t[0][:, j * P:(j + 1) * P], xt[0][:, 0:HC],
                                start=True, stop=False, skip_group_check=True,
                            )
                        for j in range(4):
                            nc.tensor.matmul(
                                state["k"][j][:, HC:CHUNK],
                                wk_t[0][:, j * P:(j + 1) * P],
                                xt[0][:, HC:CHUNK],
                                start=False, stop=False,
                                skip_group_check=True,
                            )
                        for j in range(4):
                            nc.tensor.matmul(
                                state["q"][j][:], wq_t[0][:, j * P:(j + 1) * P],
                                xt[0][:], start=True, stop=False,
                                skip_group_check=True,
                            )
                        return
                    xe.dma_start(out=xt[d][:],
                                 in_=xT[d * P:(d + 1) * P, 0:CHUNK])
                    we.dma_start(out=wk_t[d][:],
                                 in_=wkT[d * P:(d + 1) * P, :])
                    we.dma_start(out=wq_t[d][:],
                                 in_=wqT[d * P:(d + 1) * P, :])
                    for j in range(4):
                        nc.tensor.matmul(
                            state["k"][j][:], wk_t[d][:, j * P:(j + 1) * P],
                            xt[d][:], start=False, stop=(d == DT - 1),
                            skip_group_check=True,
                        )
                    for j in range(4):
                        nc.tensor.matmul(
                            state["q"][j][:], wq_t[d][:, j * P:(j + 1) * P],
                            xt[d][:], start=False, stop=(d == DT - 1),
                            skip_group_check=True,
                        )

                def wbulk():
                    for d in range(DT):
                        eng = nc.sync if d % 2 == 0 else nc.scalar
                        eng.dma_start(out=wv_t[d][:],
                                      in_=wvT[d * P:(d + 1) * P, :])
                    for h in range(NH):
                        nc.scalar.dma_start(out=woT_t[h][:],
                                            in_=woT[h * P:(h + 1) * P, :])

                def dstep(kind, w_t, d):
                    for j in range(4):
                        nc.tensor.matmul(
                            state[kind][j][:], w_t[d][:, j * P:(j + 1) * P],
                            xt[d][:], start=(d == 0), stop=(d == DT - 1),
                            skip_group_check=True,
                        )

                def vstep(d):
                    for i in range(4):
                        nc.tensor.matmul(
                            state["v"][i][:], xt[d][:, i * P:(i + 1) * P],
                            wv_t[d][:], start=(d == 0), stop=(d == DT - 1),
                            skip_group_check=True,
                        )

                def kcopy():
                    # chunk-0 copies ride the idle scalar engine (no exps
                    # yet), keeping the DVE free so the next loop's chain
                    # starts aren't gated behind a cast backlog
                    for j in range(4):
                        nc.scalar.activation(
                            kT_t[j][0][:], state["k"][j][:],
                            mybir.ActivationFunctionType.Copy)

                def qcopy():
                    qt = []
                    for j in range(4):
                        t_ = qt_pool.tile([P, CHUNK], BF16, name=f"qt{j}",
                                          tag=f"qt{j}")
                        nc.scalar.activation(
                            t_[:], state["q"][j][:],
                            mybir.ActivationFunctionType.Copy)
                        qt.append(t_)
                    qt_of[0] = qt

                def vcopy():
                    for i in range(4):
                        nc.scalar.activation(
                            v_t[i][:], state["v"][i][:],
                            mybir.ActivationFunctionType.Copy)

                kunits.append(lambda: start("k"))
                kunits.append(lambda: start("q", base=4))
                for d in range(DT):
                    kunits.append(lambda d=d: kqstep(d))
                kunits.append(wbulk)
                kunits.append(kcopy)
                kunits.append(qcopy)
                units.append(lambda: start("v"))
                for d in range(DT):
                    units.append(lambda d=d: vstep(d))
                units.append(vcopy)
                return kunits, units

            def proj_units(c):
                """Chunk-c (c>=1) projections, j-serial chains on banks
                b0/b1; one unit = 4 matmuls (one quad of d)."""
                units = []
                xt = xt_of[c]
                state = {}

                def chain_start(kind, j):
                    state[(kind, j)] = ps.tile(
                        [P, CHUNK], F32, name=f"p{kind}{j}", tag=f"b{j % 2}")

                def quad(kind, j, d0):
                    pt = state[(kind, j)]
                    for d in range(d0, d0 + 4):
                        if kind == "v":
                            nc.tensor.matmul(
                                pt[:], xt[d][:, j * P:(j + 1) * P], wv_t[d][:],
                                start=(d == 0), stop=(d == DT - 1),
                                skip_group_check=True,
                            )
                        else:
                            w_t = wk_t if kind == "k" else wq_t
                            nc.tensor.matmul(
                                pt[:], w_t[d][:, j * P:(j + 1) * P], xt[d][:],
                                start=(d == 0), stop=(d == DT - 1),
                                skip_group_check=True,
                            )

                def copy(kind, j):
                    pt = state[(kind, j)]
                    if kind == "k":
                        nc.vector.tensor_copy(kT_t[j][c][:], pt[:])
                    elif kind == "q":
                        t_ = qt_pool.tile([P, CHUNK], BF16, name=f"qt{j}",
                                          tag=f"qt{j}")
                        nc.vector.tensor_copy(t_[:], pt[:])
                        qt_of.setdefault(c, [None] * 4)[j] = t_
                    else:
                        # v copies gate the NEXT loop's chain starts (bank
                        # WAR); scalar usually has slack at phase ends while
                        # the DVE is backlogged with casts
                        nc.scalar.activation(
                            v_t[4 * c + j][:], pt[:],
                            mybir.ActivationFunctionType.Copy)

                for kind in ("k", "q", "v"):
                    for j in range(4):
                        units.append(lambda kind=kind, j=j: chain_start(kind, j))
                        for d0 in range(0, DT, 4):
                            units.append(
                                lambda kind=kind, j=j, d0=d0: quad(kind, j, d0))
                        units.append(lambda kind=kind, j=j: copy(kind, j))
                return units

            # ---------------- attention ----------------
            def attn_units(c, h):
                """Attention for (chunk c, head h).  One unit per key-tile
                group (4 score matmuls + exps + gs partial sums, with the
                pv quad of the previous group lagging one unit), then the
                diagonal group at staircase widths, flush, finalize."""
                T = 4 * c + 4
                G = T // 4
                state = {}
                units = []

                def kslice(t):
                    return kT_t[h][t // 4][:, (t % 4) * P:(t % 4 + 1) * P]

                def a_start():
                    state["oa"] = ps.tile([P, CHUNK], F32, name="oacc", tag="b7")
                    state["exps"] = [None] * T
                    # full-width running sum of all exp tiles on the vector
                    # engine; reduced over partitions by a single ones-matmul
                    # per head-chunk at finalize (a [1,512] matmul costs the
                    # PE 213ns; gpsimd cross-lane reduce measured 77us).
                    state["gst"] = gs_pool.tile([P, CHUNK], BF16, name="gst",
                                                tag="gst")

                def emit_pv(t, qs=slice(0, CHUNK), width=CHUNK, ex=None,
                            exs=None, stop=False):
                    nc.tensor.matmul(
                        state["oa"][:, qs],
                        v_t[t][:, h * P:(h + 1) * P],
                        (ex if ex is not None else state["exps"][t])[:, exs or slice(0, width)],
                        start=(t == 0), stop=stop,
                        skip_group_check=True,
                    )

                def a_group(g):
                    qt = qt_of[c]
                    for i in range(4):
                        t = 4 * g + i
                        pss = ps.tile([P, CHUNK], F32, name="pss",
                                      tag=next_score_bank())
                        nc.tensor.matmul(
                            pss[:], kslice(t), qt[h][:],
                            start=True, stop=True, skip_group_check=True,
                        )
                        e = exp_pool.tile([P, CHUNK], BF16, name="exp",
                                          tag="exp")
                        nc.scalar.activation(
                            e[:], pss[:], mybir.ActivationFunctionType.Exp,
                            scale=SCALE,
                        )
                        state["exps"][t] = e
                    gst = state["gst"]
                    ex = state["exps"]
                    if g == 0:
                        nc.vector.tensor_add(gst[:], ex[0][:], ex[1][:])
                    else:
                        nc.vector.tensor_add(gst[:], gst[:], ex[4 * g][:])
                        nc.vector.tensor_add(gst[:], gst[:], ex[4 * g + 1][:])
                    nc.vector.tensor_add(gst[:], gst[:], ex[4 * g + 2][:])
                    nc.vector.tensor_add(gst[:], gst[:], ex[4 * g + 3][:])
                    if g >= 1:
                        for i in range(4):
                            emit_pv(4 * (g - 1) + i)

                def a_group_diag(g):
                    # staircase widths: t0 full (q 0:512), t0+1 at 384
                    # (q 128:512) packed with t0+3 at 128 (q 384:512) in one
                    # psum/exp, t0+2 at 256 (q 256:512).  All mask multiplies
                    # are [128,128] against the shared lower-triangle master.
                    qt = qt_of[c]
                    t0 = 4 * g
                    # full tile t0
                    ps_a = ps.tile([P, CHUNK], F32, name="pss",
                                   tag=next_score_bank())
                    nc.tensor.matmul(ps_a[:], kslice(t0), qt[h][:],
                                     start=True, stop=True,
                                     skip_group_check=True)
                    ea = exp_pool.tile([P, CHUNK], BF16, name="exp", tag="exp")
                    nc.scalar.activation(ea[:], ps_a[:],
                                         mybir.ActivationFunctionType.Exp,
                                         scale=SCALE)
                    nc.vector.tensor_mul(ea[:, 0:P], ea[:, 0:P], master[:])
                    # packed tile: t0+1 at cols [0:384] (q 128:512),
                    # t0+3 at cols [384:512] (q 384:512)
                    ps_b = ps.tile([P, CHUNK], F32, name="pss",
                                   tag=next_score_bank())
                    nc.tensor.matmul(ps_b[:, 0:384], kslice(t0 + 1),
                                     qt[h][:, P:CHUNK],
                                     start=True, stop=True,
                                     skip_group_check=True)
                    nc.tensor.matmul(ps_b[:, 384:CHUNK], kslice(t0 + 3),
                                     qt[h][:, 384:CHUNK],
                                     start=False, stop=True,
                                     skip_group_check=True)
                    eb = exp_pool.tile([P, CHUNK], BF16, name="exp", tag="exp")
                    nc.scalar.activation(eb[:], ps_b[:],
                                         mybir.ActivationFunctionType.Exp,
                                         scale=SCALE)
                    nc.vector.tensor_mul(eb[:, 0:P], eb[:, 0:P], master[:])
                    nc.vector.tensor_mul(eb[:, 384:CHUNK], eb[:, 384:CHUNK],
                                         master[:])
                    # tile t0+2 at 256 (q 256:512)
                    ps_c = ps.tile([P, CHUNK], F32, name="pss",
                                   tag=next_score_bank())
                    nc.tensor.matmul(ps_c[:, 0:256], kslice(t0 + 2),
                                     qt[h][:, 256:CHUNK],
                                     start=True, stop=True,
                                     skip_group_check=True)
                    ec = exp_pool.tile([P, CHUNK], BF16, name="exp", tag="exp")
                    nc.scalar.activation(ec[:, 0:256], ps_c[:, 0:256],
                                         mybir.ActivationFunctionType.Exp,
                                         scale=SCALE)
                    nc.vector.tensor_mul(ec[:, 0:P], ec[:, 0:P], master[:])
                    state["ea"], state["eb"], state["ec"] = ea, eb, ec
                    # diagonal contributions to the running sum, with column
                    # realignment for the packed tiles
                    gst = state["gst"]
                    if g == 0:
                        nc.vector.tensor_copy(gst[:], ea[:])
                    else:
                        nc.vector.tensor_add(gst[:], gst[:], ea[:])
                    nc.vector.tensor_add(gst[:, P:CHUNK], gst[:, P:CHUNK],
                                         eb[:, 0:384])
                    nc.vector.tensor_add(gst[:, 256:CHUNK], gst[:, 256:CHUNK],
                                         ec[:, 0:256])
                    nc.vector.tensor_add(gst[:, 384:CHUNK], gst[:, 384:CHUNK],
                                         eb[:, 384:CHUNK])
                    if g >= 1:
                        for i in range(4):
                            emit_pv(4 * (g - 1) + i)

                def a_flush(g):
                    t0 = 4 * g
                    ea, eb, ec = state["ea"], state["eb"], state["ec"]
                    emit_pv(t0, ex=ea)
                    emit_pv(t0 + 1, qs=slice(P, CHUNK), ex=eb,
                            exs=slice(0, 384))
                    emit_pv(t0 + 2, qs=slice(256, CHUNK), ex=ec,
                            exs=slice(0, 256))
                    emit_pv(t0 + 3, qs=slice(384, CHUNK), ex=eb,
                            exs=slice(384, CHUNK), stop=True)

                def a_fin():
                    # single partition-reduce of the running exp sum; the
                    # all-ones stationary replicates the sum across all 128
                    # partitions, so no separate broadcast is needed.
                    rs = ps.tile([P, CHUNK], F32, name="rs",
                                 tag=next_score_bank())
                    nc.tensor.matmul(rs[:], ones[:], state["gst"][:],
                                     start=True, stop=True,
                                     skip_group_check=True)
                    zrec = rb_pool.tile([P, CHUNK], F32, name="zrec",
                                        tag="zr")
                    nc.vector.reciprocal_approx_fast(out=zrec[:], in_=rs[:])
                    ot = ot_pool.tile([P, CHUNK], BF16, name="ot", tag="ot")
                    nc.vector.tensor_mul(ot[:], state["oa"][:], zrec[:])
                    ots_of.setdefault(c, []).append(ot)

                units.append(a_start)
                for g in range(G - 1):
                    units.append(lambda g=g: a_group(g))
                units.append(lambda: a_group_diag(G - 1))
                units.append(lambda: a_flush(G - 1))
                units.append(a_fin)
                return units

            # ---------------- output projection ----------------
            def wo_units(c, banks=None, split_store=False):
                ssl = slice(c * CHUNK, (c + 1) * CHUNK)
                units = []

                def w_j2(j2):
                    ots = ots_of[c]
                    # banks=None -> share the score rotation (used when woven
                    # between attention groups, so psum allocation stays a
                    # single uniform round-robin at maximum depth)
                    tag = (next_score_bank() if banks is None
                           else banks[j2 % len(banks)])
                    pw = ps.tile([P, CHUNK], F32, name="pw", tag=tag)
                    for h in range(NH):
                        nc.tensor.matmul(
                            pw[:], woT_t[h][:, j2 * P:(j2 + 1) * P], ots[h][:],
                            start=(h == 0), stop=(h == NH - 1),
                            skip_group_check=True,
                        )
                    ob = osb_pool.tile([P, CHUNK], BF16, name="ob", tag="ob")
                    nc.vector.tensor_copy(ob[:], pw[:])
                    # the final chunk's stores alternate queues so the last
                    # output drain uses both DMA paths (scalar is exp-free by
                    # then); the last few split into halves across both
                    # queues to shrink the end-of-kernel transfer drain
                    if split_store and j2 >= DT - 4:
                        HC = CHUNK // 2
                        s0 = c * CHUNK
                        nc.sync.dma_start(
                            out=outT[j2 * P:(j2 + 1) * P, s0:s0 + HC],
                            in_=ob[:, 0:HC])
                        nc.scalar.dma_start(
                            out=outT[j2 * P:(j2 + 1) * P, s0 + HC:s0 + CHUNK],
                            in_=ob[:, HC:CHUNK])
                        return
                    eng = nc.scalar if (split_store and j2 % 2 == 1) else nc.sync
                    eng.dma_start(out=outT[j2 * P:(j2 + 1) * P, ssl],
                                  in_=ob[:], )

                for j2 in range(DT):
                    units.append(lambda j2=j2: w_j2(j2))
                return units

            def run(units):
                for u in units:
                    u()

            def weave(primary, filler):
                """Emit primary units with filler spread evenly between."""
                n, m = len(primary), len(filler)
                fi = 0
                for i, u in enumerate(primary):
                    u()
                    target = ((i + 1) * m) // n
                    while fi < target:
                        filler[fi]()
                        fi += 1
                while fi < m:
                    filler[fi]()
                    fi += 1

            # ---------------- software pipeline ----------------
            p0k, p0qv = proj0_units()
            run(p0k)
            weave(p0qv, xpre_units(1))
            for c in range(1, NCH):
                A = []
                for h in range(NH):
                    A.extend(attn_units(c - 1, h))
                # filler: x prefetch first, then the two-loops-back wo (its
                # casts spread between attention DVE ops instead of forming
                # a solid block), then this chunk's projections.
                F = []
                if c + 1 < NCH:
                    F.extend(xpre_units(c + 1))
                if c >= 2:
                    F.extend(wo_units(c - 2))
                F.extend(proj_units(c))
                weave(A, F)
            # tail: attn(3) woven with wo(2); a few wo(2) units are held back
            # and emitted AFTER the last head's finalize (so the finalize's
            # vector ops aren't queued behind their casts) but their matmuls
            # still feed the PE while the finalize chain completes.
            A = []
            for h in range(NH):
                A.extend(attn_units(NCH - 1, h))
            wo2 = wo_units(NCH - 2, banks=["b0", "b1"])
            weave(A[:-1], wo2[:14])
            run(A[-1:])
            run(wo2[14:])
            run(wo_units(NCH - 1, banks=["b0", "b1", "b2", "b3", "b4"],
                         split_store=True))

    nc.compile()
    return nc


_NC_CACHE = None


def _get_nc():
    global _NC_CACHE
    if _NC_CACHE is None:
        _NC_CACHE = build_kernel()
    return _NC_CACHE


def make_in_maps(x, wq, wk, wv, wo):
    bf = ml_dtypes.bfloat16
    in_maps = []
    for core in range(8):
        b, g = core // 4, core % 4
        j0 = g * JL
        in_maps.append({
            "xT": np.ascontiguousarray(x[b].T).astype(bf),
            "wqT": np.ascontiguousarray(wq[j0:j0 + JL, :].T).astype(bf),
            "wkT": np.ascontiguousarray(wk[j0:j0 + JL, :].T).astype(bf),
            "wvT": np.ascontiguousarray(wv[j0:j0 + JL, :].T).astype(bf),
            "woT": np.ascontiguousarray(wo[:, j0:j0 + JL].T).astype(bf),
        })
    return in_maps


def kernel(x, freqs_complex=None, mask=None, wq=None, wk=None, wv=None, wo=None,
           **_unused):
    x = np.asarray(x, dtype=np.float32)
    wq = np.asarray(wq, dtype=np.float32)
    wk = np.asarray(wk, dtype=np.float32)
    wv = np.asarray(wv, dtype=np.float32)
    wo = np.asarray(wo, dtype=np.float32)

    nc = _get_nc()
    in_maps = make_in_maps(x, wq, wk, wv, wo)
    res = run_bass_kernel_spmd(nc, in_maps, list(range(8)))

    out = np.zeros((B, S, D), dtype=np.float32)
    for core in range(8):
        out[core // 4] += res.results[core]["outT"].T.astype(np.float32)
    return out


# revision 37
# speedup vs baseline: 1.0198x; 1.0198x over previous
"""Multi-head causal attention (B=2, S=2048, D=2048, H=16, HD=128) on 8 TRN2
NeuronCores.

Sharding: data-parallel over batch (2 groups of 4 cores) x tensor-parallel
over heads (4 heads per core).  Each core computes q/k/v projections for its
512 columns (4 heads), causal attention for those heads, and a partial
(contraction-sharded) wo product.  The 4 partial outputs per batch are summed
on the host (the "all-reduce after wo" of the sharding hint).

Everything on-chip is computed in transposed orientation:
  xT [d, s] (host pre-transposed), qT/kT [j, s], scores^T [t, s], out^T [j2, s]
so every matmul contraction lands on the partition axis with zero on-chip
transposes.

v2 performance structure (vs the 331us v1):
  * fine-grained weave: attention group units (whose pace is set by the exp
    activation engine) are interleaved unit-by-unit with projection matmul
    units (pure PE feeders), so the PE never waits on exp.
  * softmax denominators moved off the PE: per-group column sums run as
    gpsimd partition reductions (engine ~9% busy) instead of ones-matmuls.
  * finer causal staircase: diagonal key tiles computed at widths
    512/384/128(packed)/256 instead of 512/512/256/256 (-256 PE rows and
    -256 exp columns per head-chunk); masks shrink to [128,128] multiplies.
  * all PSUM banks placed by hand via one pool with 8 tags: projections use
    2 banks (chunk-0: 4), scores rotate over 5, pv accumulator holds 1.
  * output stores ride the sync DMA queue only - the scalar engine runs
    exps exclusively after the weight prologue.
  * next chunk's x tiles prefetch on the sync queue a full phase early.

Softmax uses exp without max-subtraction (scores are O(4), exact in fp32)
with causal masking by a precomputed 0/1 lower-triangle mask applied
post-exp (exact zeros, matching the reference's exp(-1e9) == 0 underflow).
"""

import ml_dtypes
import numpy as np

import concourse.bass as bass
import concourse.tile as tile
from concourse import bacc, bass_isa, mybir
from concourse.bass_utils import run_bass_kernel_spmd

B, S, D = 2, 2048, 2048
H, HD = 16, 128
P = 128
JL = 512          # local q/k/v columns per core (4 heads)
NH = 4            # heads per core
CHUNK = 512       # s-chunk
NCH = S // CHUNK  # 4
DT = D // P       # 16 d-tiles
NT = S // P       # 16 t-tiles
SCALE = 1.0 / float(np.sqrt(HD))

F32 = mybir.dt.float32
BF16 = mybir.dt.bfloat16


def build_kernel():
    nc = bacc.Bacc("TRN2", target_bir_lowering=False, debug=False, num_devices=8)
    xT = nc.dram_tensor("xT", [D, S], BF16, kind="ExternalInput").ap()
    wqT = nc.dram_tensor("wqT", [D, JL], BF16, kind="ExternalInput").ap()
    wkT = nc.dram_tensor("wkT", [D, JL], BF16, kind="ExternalInput").ap()
    wvT = nc.dram_tensor("wvT", [D, JL], BF16, kind="ExternalInput").ap()
    woT = nc.dram_tensor("woT", [JL, D], BF16, kind="ExternalInput").ap()
    outT = nc.dram_tensor("outT", [D, S], BF16, kind="ExternalOutput").ap()

    with tile.TileContext(nc) as tc:
        with (
            tc.tile_pool(name="persist", bufs=1) as persist,
            tc.tile_pool(name="xt", bufs=2) as xt_pool,
            tc.tile_pool(name="qt", bufs=3) as qt_pool,
            tc.tile_pool(name="exp", bufs=12) as exp_pool,
            tc.tile_pool(name="gs", bufs=2) as gs_pool,
            tc.tile_pool(name="ot", bufs=10) as ot_pool,
            tc.tile_pool(name="rb", bufs=2) as rb_pool,
            tc.tile_pool(name="osb", bufs=6) as osb_pool,
            tc.tile_pool(name="ps", bufs=1, space="PSUM") as ps,
        ):
            # --- 0/1 lower-triangle mask: master[p, u] = 1 iff u >= p ---
            master_f = persist.tile([P, P], F32, name="master_f")
            nc.gpsimd.memset(master_f[:], 1.0)
            nc.gpsimd.affine_select(
                out=master_f[:], in_=master_f[:], pattern=[[1, P]],
                compare_op=mybir.AluOpType.is_ge, fill=0.0,
                base=0, channel_multiplier=-1,
            )
            master = persist.tile([P, P], BF16, name="master")
            nc.vector.tensor_copy(master[:], master_f[:])
            # all-ones [128,128] stationary: the denominator matmul then
            # lands the column sum on EVERY psum partition - the partition
            # broadcast comes free with the reduce
            ones_f = persist.tile([P, P], F32, name="ones_f")
            nc.vector.memset(ones_f[:], 1.0)
            ones = persist.tile([P, P], BF16, name="ones")
            nc.vector.tensor_copy(ones[:], ones_f[:])

            # --- resident weights + chunk-0 x: (wk_d, x_d) pairs land first,
            # alternating queues so pair d is complete after ~d/2 slots; then
            # wq, then wv, then woT. ---
            wk_t = [persist.tile([P, JL], BF16, name=f"wk{d}") for d in range(DT)]
            wq_t = [persist.tile([P, JL], BF16, name=f"wq{d}") for d in range(DT)]
            wv_t = [persist.tile([P, JL], BF16, name=f"wv{d}") for d in range(DT)]
            woT_t = [persist.tile([P, D], BF16, name=f"woT{h}") for h in range(NH)]

            xt0 = [xt_pool.tile([P, CHUNK], BF16, name=f"xt{d}", tag=f"xt{d}")
                   for d in range(DT)]

            # kT per (head, chunk): no writer/reader tile overlap across the
            # software pipeline.  v per key tile as before.
            kT_t = [[persist.tile([P, CHUNK], BF16, name=f"kT{h}_{c}")
                     for c in range(NCH)] for h in range(NH)]
            v_t = [persist.tile([P, JL], BF16, name=f"v{t}") for t in range(NT)]

            xt_of = {0: xt0}  # chunk -> xt tiles
            qt_of = {}        # chunk -> qt tiles
            ots_of = {}       # chunk -> normalized per-head attention outputs

            SCORE_BANKS = ["b2", "b3", "b4", "b5", "b6"]
            score_rr = [0]

            def next_score_bank():
                t = SCORE_BANKS[score_rr[0] % len(SCORE_BANKS)]
                score_rr[0] += 1
                return t

            # ---------------- projections ----------------
            def xpre_units(c):
                """Prefetch chunk c's x tiles on the sync queue (4 DMAs per
                unit, no PE cost)."""
                ssl = slice(c * CHUNK, (c + 1) * CHUNK)
                xt = [xt_pool.tile([P, CHUNK], BF16, name=f"xt{d}", tag=f"xt{d}")
                      for d in range(DT)]
                xt_of[c] = xt

                def issue(d0):
                    for d in range(d0, d0 + 4):
                        nc.sync.dma_start(out=xt[d][:],
                                          in_=xT[d * P:(d + 1) * P, ssl])
                return [lambda d0=d0: issue(d0) for d0 in range(0, DT, 4)]

            def proj0_units():
                """Chunk-0 projections, j-parallel over 4 banks (DMA-paced
                start); one unit = 4 matmuls for one d.  The (x_d, wk_d) DMA
                pair is issued inside unit d so each unit's semaphore wait
                covers only its own pair (a bulk prologue coalesces all 16
                DMAs onto one counter and the first matmul then waits ~4us
                for the full set).  Returns (k_units, qv_units)."""
                kunits = []
                units = []
                state = {}
                xt = xt_of[0]

                def start(kind, base=0):
                    state[kind] = [ps.tile([P, CHUNK], F32, name=f"p{kind}{j}",
                                           tag=f"b{base + j}") for j in range(4)]

                def kqstep(d):
                    # stream [x_d, wk_d, wq_d] and run BOTH the k and q
                    # matmuls for d: consumption (~1.7us/d of PE) matches the
                    # two queues' supply (~1.65us/d), so the DMA-paced head
                    # runs with the PE ~fully fed.  k accumulates on b0-b3,
                    # q on b4-b7 (scores/oa start only in loop 1).
                    xe = nc.sync if d % 2 == 0 else nc.scalar
                    we = nc.scalar if d % 2 == 0 else nc.sync
                    if d == 0:
                        # fine-grained first transfers: the opening matmul
                        # gates on wk0's j0 slice (32KB) + x0's first half
                        # (64KB) instead of two full 131KB tiles.
                        HC = CHUNK // 2
                        xe.dma_start(out=xt[0][:, 0:HC], in_=xT[0:P, 0:HC])
                        for j in range(4):
                            we.dma_start(
                                out=wk_t[0][:, j * P:(j + 1) * P],
                                in_=wkT[0:P, j * P:(j + 1) * P])
                        xe.dma_start(out=xt[0][:, HC:CHUNK],
                                     in_=xT[0:P, HC:CHUNK])
                        we.dma_start(out=wq_t[0][:], in_=wqT[0:P, :])
                        for j in range(4):
                            nc.tensor.matmul(
                                state["k"][j][:, 0:HC],
                                wk_t[0][:, j * P:(j + 1) * P], xt[0][:, 0:HC],
                                start=True, stop=False, skip_group_check=True,
                            )
                        for j in range(4):
                            nc.tensor.matmul(
                                state["k"][j][:, HC:CHUNK],
                                wk_t[0][:, j * P:(j + 1) * P],
                                xt[0][:, HC:CHUNK],
                                start=False, stop=False,
                                skip_group_check=True,
                            )
                        for j in range(4):
                            nc.tensor.matmul(
                                state["q"][j][:], wq_t[0][:, j * P:(j + 1) * P],
                                xt[0][:], start=True, stop=False,
                                skip_group_check=True,
                            )
                        return
                    xe.dma_start(out=xt[d][:],
                                 in_=xT[d * P:(d + 1) * P, 0:CHUNK])
                    we.dma_start(out=wk_t[d][:],
                                 in_=wkT[d * P:(d + 1) * P, :])
                    we.dma_start(out=wq_t[d][:],
                                 in_=wqT[d * P:(d + 1) * P, :])
                    for j in range(4):
                        nc.tensor.matmul(
                            state["k"][j][:], wk_t[d][:, j * P:(j + 1) * P],
                            xt[d][:], start=False, stop=(d == DT - 1),
                            skip_group_check=True,
                        )
                    for j in range(4):
                        nc.tensor.matmul(
                            state["q"][j][:], wq_t[d][:, j * P:(j + 1) * P],
                            xt[d][:], start=False, stop=(d == DT - 1),
                            skip_group_check=True,
                        )

                def wbulk():
                    for d in range(DT):
                        eng = nc.sync if d % 2 == 0 else nc.scalar
                        eng.dma_start(out=wv_t[d][:],
                                      in_=wvT[d * P:(d + 1) * P, :])
                    for h in range(NH):
                        nc.scalar.dma_start(out=woT_t[h][:],
                                            in_=woT[h * P:(h + 1) * P, :])

                def dstep(kind, w_t, d):
                    for j in range(4):
                        nc.tensor.matmul(
                            state[kind][j][:], w_t[d][:, j * P:(j + 1) * P],
                            xt[d][:], start=(d == 0), stop=(d == DT - 1),
                            skip_group_check=True,
                        )

                def vstep(d):
                    for i in range(4):
                        nc.tensor.matmul(
                            state["v"][i][:], xt[d][:, i * P:(i + 1) * P],
                            wv_t[d][:], start=(d == 0), stop=(d == DT - 1),
                            skip_group_check=True,
                        )

                def kcopy():
                    # chunk-0 copies ride the idle scalar engine (no exps
                    # yet), keeping the DVE free so the next loop's chain
                    # starts aren't gated behind a cast backlog
                    for j in range(4):
                        nc.scalar.activation(
                            kT_t[j][0][:], state["k"][j][:],
                            mybir.ActivationFunctionType.Copy)

                def qcopy():
                    qt = []
                    for j in range(4):
                        t_ = qt_pool.tile([P, CHUNK], BF16, name=f"qt{j}",
                                          tag=f"qt{j}")
                        nc.scalar.activation(
                            t_[:], state["q"][j][:],
                            mybir.ActivationFunctionType.Copy)
                        qt.append(t_)
                    qt_of[0] = qt

                def vcopy():
                    for i in range(4):
                        nc.scalar.activation(
                            v_t[i][:], state["v"][i][:],
                            mybir.ActivationFunctionType.Copy)

                kunits.append(lambda: start("k"))
                kunits.append(lambda: start("q", base=4))
                for d in range(DT):
                    kunits.append(lambda d=d: kqstep(d))
                kunits.append(wbulk)
                kunits.append(kcopy)
                kunits.append(qcopy)
                units.append(lambda: start("v"))
                for d in range(DT):
                    units.append(lambda d=d: vstep(d))
                units.append(vcopy)
                return kunits, units

            def proj_units(c):
                """Chunk-c (c>=1) projections, j-serial chains on banks
                b0/b1; one unit = 4 matmuls (one quad of d)."""
                units = []
                xt = xt_of[c]
                state = {}

                def chain_start(kind, j):
                    state[(kind, j)] = ps.tile(
                        [P, CHUNK], F32, name=f"p{kind}{j}", tag=f"b{j % 2}")

                def quad(kind, j, d0):
                    pt = state[(kind, j)]
                    for d in range(d0, d0 + 4):
                        if kind == "v":
                            nc.tensor.matmul(
                                pt[:], xt[d][:, j * P:(j + 1) * P], wv_t[d][:],
                                start=(d == 0), stop=(d == DT - 1),
                                skip_group_check=True,
                            )
                        else:
                            w_t = wk_t if kind == "k" else wq_t
                            nc.tensor.matmul(
                                pt[:], w_t[d][:, j * P:(j + 1) * P], xt[d][:],
                                start=(d == 0), stop=(d == DT - 1),
                                skip_group_check=True,
                            )

                def copy(kind, j):
                    pt = state[(kind, j)]
                    if kind == "k":
                        nc.vector.tensor_copy(kT_t[j][c][:], pt[:])
                    elif kind == "q":
                        t_ = qt_pool.tile([P, CHUNK], BF16, name=f"qt{j}",
                                          tag=f"qt{j}")
                        nc.vector.tensor_copy(t_[:], pt[:])
                        qt_of.setdefault(c, [None] * 4)[j] = t_
                    else:
                        nc.vector.tensor_copy(v_t[4 * c + j][:], pt[:])

                for kind in ("k", "q", "v"):
                    for j in range(4):
                        units.append(lambda kind=kind, j=j: chain_start(kind, j))
                        for d0 in range(0, DT, 4):
                            units.append(
                                lambda kind=kind, j=j, d0=d0: quad(kind, j, d0))
                        units.append(lambda kind=kind, j=j: copy(kind, j))
                return units

            # ---------------- attention ----------------
            def attn_units(c, h):
                """Attention for (chunk c, head h).  One unit per key-tile
                group (4 score matmuls + exps + gs partial sums, with the
                pv quad of the previous group lagging one unit), then the
                diagonal group at staircase widths, flush, finalize."""
                T = 4 * c + 4
                G = T // 4
                state = {}
                units = []

                def kslice(t):
                    return kT_t[h][t // 4][:, (t % 4) * P:(t % 4 + 1) * P]

                def a_start():
                    state["oa"] = ps.tile([P, CHUNK], F32, name="oacc", tag="b7")
                    state["exps"] = [None] * T
                    # full-width running sum of all exp tiles on the vector
                    # engine; reduced over partitions by a single ones-matmul
                    # per head-chunk at finalize (a [1,512] matmul costs the
                    # PE 213ns; gpsimd cross-lane reduce measured 77us).
                    state["gst"] = gs_pool.tile([P, CHUNK], BF16, name="gst",
                                                tag="gst")

                def emit_pv(t, qs=slice(0, CHUNK), width=CHUNK, ex=None,
                            exs=None, stop=False):
                    nc.tensor.matmul(
                        state["oa"][:, qs],
                        v_t[t][:, h * P:(h + 1) * P],
                        (ex if ex is not None else state["exps"][t])[:, exs or slice(0, width)],
                        start=(t == 0), stop=stop,
                        skip_group_check=True,
                    )

                def a_group(g):
                    qt = qt_of[c]
                    for i in range(4):
                        t = 4 * g + i
                        pss = ps.tile([P, CHUNK], F32, name="pss",
                                      tag=next_score_bank())
                        nc.tensor.matmul(
                            pss[:], kslice(t), qt[h][:],
                            start=True, stop=True, skip_group_check=True,
                        )
                        e = exp_pool.tile([P, CHUNK], BF16, name="exp",
                                          tag="exp")
                        nc.scalar.activation(
                            e[:], pss[:], mybir.ActivationFunctionType.Exp,
                            scale=SCALE,
                        )
                        state["exps"][t] = e
                    gst = state["gst"]
                    ex = state["exps"]
                    if g == 0:
                        nc.vector.tensor_add(gst[:], ex[0][:], ex[1][:])
                    else:
                        nc.vector.tensor_add(gst[:], gst[:], ex[4 * g][:])
                        nc.vector.tensor_add(gst[:], gst[:], ex[4 * g + 1][:])
                    nc.vector.tensor_add(gst[:], gst[:], ex[4 * g + 2][:])
                    nc.vector.tensor_add(gst[:], gst[:], ex[4 * g + 3][:])
                    if g >= 1:
                        for i in range(4):
                            emit_pv(4 * (g - 1) + i)

                def a_group_diag(g):
                    # staircase widths: t0 full (q 0:512), t0+1 at 384
                    # (q 128:512) packed with t0+3 at 128 (q 384:512) in one
                    # psum/exp, t0+2 at 256 (q 256:512).  All mask multiplies
                    # are [128,128] against the shared lower-triangle master.
                    qt = qt_of[c]
                    t0 = 4 * g
                    # full tile t0
                    ps_a = ps.tile([P, CHUNK], F32, name="pss",
                                   tag=next_score_bank())
                    nc.tensor.matmul(ps_a[:], kslice(t0), qt[h][:],
                                     start=True, stop=True,
                                     skip_group_check=True)
                    ea = exp_pool.tile([P, CHUNK], BF16, name="exp", tag="exp")
                    nc.scalar.activation(ea[:], ps_a[:],
                                         mybir.ActivationFunctionType.Exp,
                                         scale=SCALE)
                    nc.vector.tensor_mul(ea[:, 0:P], ea[:, 0:P], master[:])
                    # packed tile: t0+1 at cols [0:384] (q 128:512),
                    # t0+3 at cols [384:512] (q 384:512)
                    ps_b = ps.tile([P, CHUNK], F32, name="pss",
                                   tag=next_score_bank())
                    nc.tensor.matmul(ps_b[:, 0:384], kslice(t0 + 1),
                                     qt[h][:, P:CHUNK],
                                     start=True, stop=True,
                                     skip_group_check=True)
                    nc.tensor.matmul(ps_b[:, 384:CHUNK], kslice(t0 + 3),
                                     qt[h][:, 384:CHUNK],
                                     start=False, stop=True,
                                     skip_group_check=True)
                    eb = exp_pool.tile([P, CHUNK], BF16, name="exp", tag="exp")
                    nc.scalar.activation(eb[:], ps_b[:],
                                         mybir.ActivationFunctionType.Exp,
                                         scale=SCALE)
                    nc.vector.tensor_mul(eb[:, 0:P], eb[:, 0:P], master[:])
                    nc.vector.tensor_mul(eb[:, 384:CHUNK], eb[:, 384:CHUNK],
                                         master[:])
                    # tile t0+2 at 256 (q 256:512)
                    ps_c = ps.tile([P, CHUNK], F32, name="pss",
                                   tag=next_score_bank())
                    nc.tensor.matmul(ps_c[:, 0:256], kslice(t0 + 2),
                                     qt[h][:, 256:CHUNK],
                                     start=True, stop=True,
                                     skip_group_check=True)
                    ec = exp_pool.tile([P, CHUNK], BF16, name="exp", tag="exp")
                    nc.scalar.activation(ec[:, 0:256], ps_c[:, 0:256],
                                         mybir.ActivationFunctionType.Exp,
                                         scale=SCALE)
                    nc.vector.tensor_mul(ec[:, 0:P], ec[:, 0:P], master[:])
                    state["ea"], state["eb"], state["ec"] = ea, eb, ec
                    # diagonal contributions to the running sum, with column
                    # realignment for the packed tiles
                    gst = state["gst"]
                    if g == 0:
                        nc.vector.tensor_copy(gst[:], ea[:])
                    else:
                        nc.vector.tensor_add(gst[:], gst[:], ea[:])
                    nc.vector.tensor_add(gst[:, P:CHUNK], gst[:, P:CHUNK],
                                         eb[:, 0:384])
                    nc.vector.tensor_add(gst[:, 256:CHUNK], gst[:, 256:CHUNK],
                                         ec[:, 0:256])
                    nc.vector.tensor_add(gst[:, 384:CHUNK], gst[:, 384:CHUNK],
                                         eb[:, 384:CHUNK])
                    if g >= 1:
                        for i in range(4):
                            emit_pv(4 * (g - 1) + i)

                def a_flush(g):
                    t0 = 4 * g
                    ea, eb, ec = state["ea"], state["eb"], state["ec"]
                    emit_pv(t0, ex=ea)
                    emit_pv(t0 + 1, qs=slice(P, CHUNK), ex=eb,
                            exs=slice(0, 384))
                    emit_pv(t0 + 2, qs=slice(256, CHUNK), ex=ec,
                            exs=slice(0, 256))
                    emit_pv(t0 + 3, qs=slice(384, CHUNK), ex=eb,
                            exs=slice(384, CHUNK), stop=True)

                def a_fin():
                    # single partition-reduce of the running exp sum; the
                    # all-ones stationary replicates the sum across all 128
                    # partitions, so no separate broadcast is needed.
                    rs = ps.tile([P, CHUNK], F32, name="rs",
                                 tag=next_score_bank())
                    nc.tensor.matmul(rs[:], ones[:], state["gst"][:],
                                     start=True, stop=True,
                                     skip_group_check=True)
                    zrec = rb_pool.tile([P, CHUNK], F32, name="zrec",
                                        tag="zr")
                    nc.vector.reciprocal_approx_fast(out=zrec[:], in_=rs[:])
                    ot = ot_pool.tile([P, CHUNK], BF16, name="ot", tag="ot")
                    nc.vector.tensor_mul(ot[:], state["oa"][:], zrec[:])
                    ots_of.setdefault(c, []).append(ot)

                units.append(a_start)
                for g in range(G - 1):
                    units.append(lambda g=g: a_group(g))
                units.append(lambda: a_group_diag(G - 1))
                units.append(lambda: a_flush(G - 1))
                units.append(a_fin)
                return units

            # ---------------- output projection ----------------
            def wo_units(c, banks=None, split_store=False):
                ssl = slice(c * CHUNK, (c + 1) * CHUNK)
                units = []

                def w_j2(j2):
                    ots = ots_of[c]
                    # banks=None -> share the score rotation (used when woven
                    # between attention groups, so psum allocation stays a
                    # single uniform round-robin at maximum depth)
                    tag = (next_score_bank() if banks is None
                           else banks[j2 % len(banks)])
                    pw = ps.tile([P, CHUNK], F32, name="pw", tag=tag)
                    for h in range(NH):
                        nc.tensor.matmul(
                            pw[:], woT_t[h][:, j2 * P:(j2 + 1) * P], ots[h][:],
                            start=(h == 0), stop=(h == NH - 1),
                            skip_group_check=True,
                        )
                    ob = osb_pool.tile([P, CHUNK], BF16, name="ob", tag="ob")
                    nc.vector.tensor_copy(ob[:], pw[:])
                    # the final chunk's stores alternate queues so the last
                    # output drain uses both DMA paths (scalar is exp-free by
                    # then); the last few split into halves across both
                    # queues to shrink the end-of-kernel transfer drain
                    if split_store and j2 >= DT - 4:
                        HC = CHUNK // 2
                        s0 = c * CHUNK
                        nc.sync.dma_start(
                            out=outT[j2 * P:(j2 + 1) * P, s0:s0 + HC],
                            in_=ob[:, 0:HC])
                        nc.scalar.dma_start(
                            out=outT[j2 * P:(j2 + 1) * P, s0 + HC:s0 + CHUNK],
                            in_=ob[:, HC:CHUNK])
                        return
                    eng = nc.scalar if (split_store and j2 % 2 == 1) else nc.sync
                    eng.dma_start(out=outT[j2 * P:(j2 + 1) * P, ssl],
                                  in_=ob[:], )

                for j2 in range(DT):
                    units.append(lambda j2=j2: w_j2(j2))
                return units

            def run(units):
                for u in units:
                    u()

            def weave(primary, filler):
                """Emit primary units with filler spread evenly between."""
                n, m = len(primary), len(filler)
                fi = 0
                for i, u in enumerate(primary):
                    u()
                    target = ((i + 1) * m) // n
                    while fi < target:
                        filler[fi]()
                        fi += 1
                while fi < m:
                    filler[fi]()
                    fi += 1

            # ---------------- software pipeline ----------------
            p0k, p0qv = proj0_units()
            run(p0k)
            weave(p0qv, xpre_units(1))
            for c in range(1, NCH):
                A = []
                for h in range(NH):
                    A.extend(attn_units(c - 1, h))
                # filler: x prefetch first, then the two-loops-back wo (its
                # casts spread between attention DVE ops instead of forming
                # a solid block), then this chunk's projections.
                F = []
                if c + 1 < NCH:
                    F.extend(xpre_units(c + 1))
                if c >= 2:
                    F.extend(wo_units(c - 2))
                F.extend(proj_units(c))
                weave(A, F)
            # tail: attn(3) woven with wo(2); a few wo(2) units are held back
            # and emitted AFTER the last head's finalize (so the finalize's
            # vector ops aren't queued behind their casts) but their matmuls
            # still feed the PE while the finalize chain completes.
            A = []
            for h in range(NH):
                A.extend(attn_units(NCH - 1, h))
            wo2 = wo_units(NCH - 2, banks=["b0", "b1"])
            weave(A[:-1], wo2[:14])
            run(A[-1:])
            run(wo2[14:])
            run(wo_units(NCH - 1, banks=["b0", "b1", "b2", "b3", "b4"],
                         split_store=True))

    nc.compile()
    return nc


_NC_CACHE = None


def _get_nc():
    global _NC_CACHE
    if _NC_CACHE is None:
        _NC_CACHE = build_kernel()
    return _NC_CACHE


def make_in_maps(x, wq, wk, wv, wo):
    bf = ml_dtypes.bfloat16
    in_maps = []
    for core in range(8):
        b, g = core // 4, core % 4
        j0 = g * JL
        in_maps.append({
            "xT": np.ascontiguousarray(x[b].T).astype(bf),
            "wqT": np.ascontiguousarray(wq[j0:j0 + JL, :].T).astype(bf),
            "wkT": np.ascontiguousarray(wk[j0:j0 + JL, :].T).astype(bf),
            "wvT": np.ascontiguousarray(wv[j0:j0 + JL, :].T).astype(bf),
            "woT": np.ascontiguousarray(wo[:, j0:j0 + JL].T).astype(bf),
        })
    return in_maps


def kernel(x, freqs_complex=None, mask=None, wq=None, wk=None, wv=None, wo=None,
           **_unused):
    x = np.asarray(x, dtype=np.float32)
    wq = np.asarray(wq, dtype=np.float32)
    wk = np.asarray(wk, dtype=np.float32)
    wv = np.asarray(wv, dtype=np.float32)
    wo = np.asarray(wo, dtype=np.float32)

    nc = _get_nc()
    in_maps = make_in_maps(x, wq, wk, wv, wo)
    res = run_bass_kernel_spmd(nc, in_maps, list(range(8)))

    out = np.zeros((B, S, D), dtype=np.float32)
    for core in range(8):
        out[core // 4] += res.results[core]["outT"].T.astype(np.float32)
    return out


# revision 38
# speedup vs baseline: 1.0486x; 1.0283x over previous
"""Multi-head causal attention (B=2, S=2048, D=2048, H=16, HD=128) on 8 TRN2
NeuronCores.

Sharding: data-parallel over batch (2 groups of 4 cores) x tensor-parallel
over heads (4 heads per core).  Each core computes q/k/v projections for its
512 columns (4 heads), causal attention for those heads, and a partial
(contraction-sharded) wo product.  The 4 partial outputs per batch are summed
on the host (the "all-reduce after wo" of the sharding hint).

Everything on-chip is computed in transposed orientation:
  xT [d, s] (host pre-transposed), qT/kT [j, s], scores^T [t, s], out^T [j2, s]
so every matmul contraction lands on the partition axis with zero on-chip
transposes.

v2 performance structure (vs the 331us v1):
  * fine-grained weave: attention group units (whose pace is set by the exp
    activation engine) are interleaved unit-by-unit with projection matmul
    units (pure PE feeders), so the PE never waits on exp.
  * softmax denominators moved off the PE: per-group column sums run as
    gpsimd partition reductions (engine ~9% busy) instead of ones-matmuls.
  * finer causal staircase: diagonal key tiles computed at widths
    512/384/128(packed)/256 instead of 512/512/256/256 (-256 PE rows and
    -256 exp columns per head-chunk); masks shrink to [128,128] multiplies.
  * all PSUM banks placed by hand via one pool with 8 tags: projections use
    2 banks (chunk-0: 4), scores rotate over 5, pv accumulator holds 1.
  * output stores ride the sync DMA queue only - the scalar engine runs
    exps exclusively after the weight prologue.
  * next chunk's x tiles prefetch on the sync queue a full phase early.

Softmax uses exp without max-subtraction (scores are O(4), exact in fp32)
with causal masking by a precomputed 0/1 lower-triangle mask applied
post-exp (exact zeros, matching the reference's exp(-1e9) == 0 underflow).
"""

import ml_dtypes
import numpy as np

import concourse.bass as bass
import concourse.tile as tile
from concourse import bacc, bass_isa, mybir
from concourse.bass_utils import run_bass_kernel_spmd

B, S, D = 2, 2048, 2048
H, HD = 16, 128
P = 128
JL = 512          # local q/k/v columns per core (4 heads)
NH = 4            # heads per core
CHUNK = 512       # s-chunk
NCH = S // CHUNK  # 4
DT = D // P       # 16 d-tiles
NT = S // P       # 16 t-tiles
SCALE = 1.0 / float(np.sqrt(HD))

F32 = mybir.dt.float32
BF16 = mybir.dt.bfloat16


def build_kernel():
    nc = bacc.Bacc("TRN2", target_bir_lowering=False, debug=False, num_devices=8)
    xT = nc.dram_tensor("xT", [D, S], BF16, kind="ExternalInput").ap()
    wqT = nc.dram_tensor("wqT", [D, JL], BF16, kind="ExternalInput").ap()
    wkT = nc.dram_tensor("wkT", [D, JL], BF16, kind="ExternalInput").ap()
    wvT = nc.dram_tensor("wvT", [D, JL], BF16, kind="ExternalInput").ap()
    woT = nc.dram_tensor("woT", [JL, D], BF16, kind="ExternalInput").ap()
    outT = nc.dram_tensor("outT", [D, S], BF16, kind="ExternalOutput").ap()

    with tile.TileContext(nc) as tc:
        with (
            tc.tile_pool(name="persist", bufs=1) as persist,
            tc.tile_pool(name="xt", bufs=2) as xt_pool,
            tc.tile_pool(name="qt", bufs=3) as qt_pool,
            tc.tile_pool(name="exp", bufs=12) as exp_pool,
            tc.tile_pool(name="gs", bufs=2) as gs_pool,
            tc.tile_pool(name="ot", bufs=10) as ot_pool,
            tc.tile_pool(name="rb", bufs=2) as rb_pool,
            tc.tile_pool(name="osb", bufs=6) as osb_pool,
            tc.tile_pool(name="ps", bufs=1, space="PSUM") as ps,
        ):
            # --- 0/1 lower-triangle mask: master[p, u] = 1 iff u >= p ---
            master_f = persist.tile([P, P], F32, name="master_f")
            nc.gpsimd.memset(master_f[:], 1.0)
            nc.gpsimd.affine_select(
                out=master_f[:], in_=master_f[:], pattern=[[1, P]],
                compare_op=mybir.AluOpType.is_ge, fill=0.0,
                base=0, channel_multiplier=-1,
            )
            master = persist.tile([P, P], BF16, name="master")
            nc.vector.tensor_copy(master[:], master_f[:])
            # all-ones [128,128] stationary: the denominator matmul then
            # lands the column sum on EVERY psum partition - the partition
            # broadcast comes free with the reduce
            ones_f = persist.tile([P, P], F32, name="ones_f")
            nc.vector.memset(ones_f[:], 1.0)
            ones = persist.tile([P, P], BF16, name="ones")
            nc.vector.tensor_copy(ones[:], ones_f[:])

            # --- resident weights + chunk-0 x: (wk_d, x_d) pairs land first,
            # alternating queues so pair d is complete after ~d/2 slots; then
            # wq, then wv, then woT. ---
            wk_t = [persist.tile([P, JL], BF16, name=f"wk{d}") for d in range(DT)]
            wq_t = [persist.tile([P, JL], BF16, name=f"wq{d}") for d in range(DT)]
            wv_t = [persist.tile([P, JL], BF16, name=f"wv{d}") for d in range(DT)]
            woT_t = [persist.tile([P, D], BF16, name=f"woT{h}") for h in range(NH)]

            xt0 = [xt_pool.tile([P, CHUNK], BF16, name=f"xt{d}", tag=f"xt{d}")
                   for d in range(DT)]

            # kT per (head, chunk): no writer/reader tile overlap across the
            # software pipeline.  v per key tile as before.
            kT_t = [[persist.tile([P, CHUNK], BF16, name=f"kT{h}_{c}")
                     for c in range(NCH)] for h in range(NH)]
            v_t = [persist.tile([P, JL], BF16, name=f"v{t}") for t in range(NT)]

            xt_of = {0: xt0}  # chunk -> xt tiles
            qt_of = {}        # chunk -> qt tiles
            ots_of = {}       # chunk -> normalized per-head attention outputs

            SCORE_BANKS = ["b2", "b3", "b4", "b5", "b6"]
            score_rr = [0]

            def next_score_bank():
                t = SCORE_BANKS[score_rr[0] % len(SCORE_BANKS)]
                score_rr[0] += 1
                return t

            # ---------------- projections ----------------
            def xpre_units(c):
                """Prefetch chunk c's x tiles on the sync queue (4 DMAs per
                unit, no PE cost)."""
                ssl = slice(c * CHUNK, (c + 1) * CHUNK)
                xt = [xt_pool.tile([P, CHUNK], BF16, name=f"xt{d}", tag=f"xt{d}")
                      for d in range(DT)]
                xt_of[c] = xt

                def issue(d0):
                    for d in range(d0, d0 + 4):
                        nc.sync.dma_start(out=xt[d][:],
                                          in_=xT[d * P:(d + 1) * P, ssl])
                return [lambda d0=d0: issue(d0) for d0 in range(0, DT, 4)]

            def proj0_units():
                """Chunk-0 projections, j-parallel over 4 banks (DMA-paced
                start); one unit = 4 matmuls for one d.  The (x_d, wk_d) DMA
                pair is issued inside unit d so each unit's semaphore wait
                covers only its own pair (a bulk prologue coalesces all 16
                DMAs onto one counter and the first matmul then waits ~4us
                for the full set).  Returns (k_units, qv_units)."""
                kunits = []
                units = []
                state = {}
                xt = xt_of[0]

                def start(kind, base=0):
                    state[kind] = [ps.tile([P, CHUNK], F32, name=f"p{kind}{j}",
                                           tag=f"b{base + j}") for j in range(4)]

                def kqstep(d):
                    # stream [x_d, wk_d, wq_d] and run BOTH the k and q
                    # matmuls for d: consumption (~1.7us/d of PE) matches the
                    # two queues' supply (~1.65us/d), so the DMA-paced head
                    # runs with the PE ~fully fed.  k accumulates on b0-b3,
                    # q on b4-b7 (scores/oa start only in loop 1).
                    xe = nc.sync if d % 2 == 0 else nc.scalar
                    we = nc.scalar if d % 2 == 0 else nc.sync
                    if d == 0:
                        # fine-grained first transfers: the opening matmul
                        # gates on wk0's j0 slice (32KB) + x0's first half
                        # (64KB) instead of two full 131KB tiles.
                        HC = CHUNK // 2
                        xe.dma_start(out=xt[0][:, 0:HC], in_=xT[0:P, 0:HC])
                        for j in range(4):
                            we.dma_start(
                                out=wk_t[0][:, j * P:(j + 1) * P],
                                in_=wkT[0:P, j * P:(j + 1) * P])
                        xe.dma_start(out=xt[0][:, HC:CHUNK],
                                     in_=xT[0:P, HC:CHUNK])
                        we.dma_start(out=wq_t[0][:], in_=wqT[0:P, :])
                        for j in range(4):
                            nc.tensor.matmul(
                                state["k"][j][:, 0:HC],
                                wk_t[0][:, j * P:(j + 1) * P], xt[0][:, 0:HC],
                                start=True, stop=False, skip_group_check=True,
                            )
                        for j in range(4):
                            nc.tensor.matmul(
                                state["k"][j][:, HC:CHUNK],
                                wk_t[0][:, j * P:(j + 1) * P],
                                xt[0][:, HC:CHUNK],
                                start=False, stop=False,
                                skip_group_check=True,
                            )
                        for j in range(4):
                            nc.tensor.matmul(
                                state["q"][j][:], wq_t[0][:, j * P:(j + 1) * P],
                                xt[0][:], start=True, stop=False,
                                skip_group_check=True,
                            )
                        return
                    xe.dma_start(out=xt[d][:],
                                 in_=xT[d * P:(d + 1) * P, 0:CHUNK])
                    we.dma_start(out=wk_t[d][:],
                                 in_=wkT[d * P:(d + 1) * P, :])
                    we.dma_start(out=wq_t[d][:],
                                 in_=wqT[d * P:(d + 1) * P, :])
                    for j in range(4):
                        nc.tensor.matmul(
                            state["k"][j][:], wk_t[d][:, j * P:(j + 1) * P],
                            xt[d][:], start=False, stop=(d == DT - 1),
                            skip_group_check=True,
                        )
                    for j in range(4):
                        nc.tensor.matmul(
                            state["q"][j][:], wq_t[d][:, j * P:(j + 1) * P],
                            xt[d][:], start=False, stop=(d == DT - 1),
                            skip_group_check=True,
                        )

                def wbulk():
                    for d in range(DT):
                        eng = nc.sync if d % 2 == 0 else nc.scalar
                        eng.dma_start(out=wv_t[d][:],
                                      in_=wvT[d * P:(d + 1) * P, :])
                    for h in range(NH):
                        nc.scalar.dma_start(out=woT_t[h][:],
                                            in_=woT[h * P:(h + 1) * P, :])

                def dstep(kind, w_t, d):
                    for j in range(4):
                        nc.tensor.matmul(
                            state[kind][j][:], w_t[d][:, j * P:(j + 1) * P],
                            xt[d][:], start=(d == 0), stop=(d == DT - 1),
                            skip_group_check=True,
                        )

                def vstep(d):
                    for i in range(4):
                        nc.tensor.matmul(
                            state["v"][i][:], xt[d][:, i * P:(i + 1) * P],
                            wv_t[d][:], start=(d == 0), stop=(d == DT - 1),
                            skip_group_check=True,
                        )

                def kcopy():
                    for j in range(4):
                        nc.vector.tensor_copy(kT_t[j][0][:], state["k"][j][:])

                def qcopy():
                    qt = []
                    for j in range(4):
                        t_ = qt_pool.tile([P, CHUNK], BF16, name=f"qt{j}",
                                          tag=f"qt{j}")
                        nc.vector.tensor_copy(t_[:], state["q"][j][:])
                        qt.append(t_)
                    qt_of[0] = qt

                def vcopy():
                    for i in range(4):
                        nc.vector.tensor_copy(v_t[i][:], state["v"][i][:])

                kunits.append(lambda: start("k"))
                kunits.append(lambda: start("q", base=4))
                for d in range(DT):
                    kunits.append(lambda d=d: kqstep(d))
                kunits.append(wbulk)
                kunits.append(kcopy)
                kunits.append(qcopy)
                units.append(lambda: start("v"))
                for d in range(DT):
                    units.append(lambda d=d: vstep(d))
                units.append(vcopy)
                return kunits, units

            def proj_units(c):
                """Chunk-c (c>=1) projections, j-serial chains on banks
                b0/b1; one unit = 4 matmuls (one quad of d)."""
                units = []
                xt = xt_of[c]
                state = {}

                def chain_start(kind, j):
                    state[(kind, j)] = ps.tile(
                        [P, CHUNK], F32, name=f"p{kind}{j}", tag=f"b{j % 2}")

                def quad(kind, j, d0):
                    pt = state[(kind, j)]
                    for d in range(d0, d0 + 4):
                        if kind == "v":
                            nc.tensor.matmul(
                                pt[:], xt[d][:, j * P:(j + 1) * P], wv_t[d][:],
                                start=(d == 0), stop=(d == DT - 1),
                                skip_group_check=True,
                            )
                        else:
                            w_t = wk_t if kind == "k" else wq_t
                            nc.tensor.matmul(
                                pt[:], w_t[d][:, j * P:(j + 1) * P], xt[d][:],
                                start=(d == 0), stop=(d == DT - 1),
                                skip_group_check=True,
                            )

                def copy(kind, j):
                    pt = state[(kind, j)]
                    if kind == "k":
                        nc.vector.tensor_copy(kT_t[j][c][:], pt[:])
                    elif kind == "q":
                        t_ = qt_pool.tile([P, CHUNK], BF16, name=f"qt{j}",
                                          tag=f"qt{j}")
                        nc.vector.tensor_copy(t_[:], pt[:])
                        qt_of.setdefault(c, [None] * 4)[j] = t_
                    else:
                        nc.vector.tensor_copy(v_t[4 * c + j][:], pt[:])

                for kind in ("k", "q", "v"):
                    for j in range(4):
                        units.append(lambda kind=kind, j=j: chain_start(kind, j))
                        for d0 in range(0, DT, 4):
                            units.append(
                                lambda kind=kind, j=j, d0=d0: quad(kind, j, d0))
                        units.append(lambda kind=kind, j=j: copy(kind, j))
                return units

            # ---------------- attention ----------------
            def attn_units(c, h):
                """Attention for (chunk c, head h).  One unit per key-tile
                group (4 score matmuls + exps + gs partial sums, with the
                pv quad of the previous group lagging one unit), then the
                diagonal group at staircase widths, flush, finalize."""
                T = 4 * c + 4
                G = T // 4
                state = {}
                units = []

                def kslice(t):
                    return kT_t[h][t // 4][:, (t % 4) * P:(t % 4 + 1) * P]

                def a_start():
                    state["oa"] = ps.tile([P, CHUNK], F32, name="oacc", tag="b7")
                    state["exps"] = [None] * T
                    # full-width running sum of all exp tiles on the vector
                    # engine; reduced over partitions by a single ones-matmul
                    # per head-chunk at finalize (a [1,512] matmul costs the
                    # PE 213ns; gpsimd cross-lane reduce measured 77us).
                    state["gst"] = gs_pool.tile([P, CHUNK], BF16, name="gst",
                                                tag="gst")

                def emit_pv(t, qs=slice(0, CHUNK), width=CHUNK, ex=None,
                            exs=None, stop=False):
                    nc.tensor.matmul(
                        state["oa"][:, qs],
                        v_t[t][:, h * P:(h + 1) * P],
                        (ex if ex is not None else state["exps"][t])[:, exs or slice(0, width)],
                        start=(t == 0), stop=stop,
                        skip_group_check=True,
                    )

                def a_group(g):
                    qt = qt_of[c]
                    for i in range(4):
                        t = 4 * g + i
                        pss = ps.tile([P, CHUNK], F32, name="pss",
                                      tag=next_score_bank())
                        nc.tensor.matmul(
                            pss[:], kslice(t), qt[h][:],
                            start=True, stop=True, skip_group_check=True,
                        )
                        e = exp_pool.tile([P, CHUNK], BF16, name="exp",
                                          tag="exp")
                        nc.scalar.activation(
                            e[:], pss[:], mybir.ActivationFunctionType.Exp,
                            scale=SCALE,
                        )
                        state["exps"][t] = e
                    gst = state["gst"]
                    ex = state["exps"]
                    if g == 0:
                        nc.vector.tensor_add(gst[:], ex[0][:], ex[1][:])
                    else:
                        nc.vector.tensor_add(gst[:], gst[:], ex[4 * g][:])
                        nc.vector.tensor_add(gst[:], gst[:], ex[4 * g + 1][:])
                    nc.vector.tensor_add(gst[:], gst[:], ex[4 * g + 2][:])
                    nc.vector.tensor_add(gst[:], gst[:], ex[4 * g + 3][:])
                    if g >= 1:
                        for i in range(4):
                            emit_pv(4 * (g - 1) + i)

                def a_group_diag(g):
                    # staircase widths: t0 full (q 0:512), t0+1 at 384
                    # (q 128:512) packed with t0+3 at 128 (q 384:512) in one
                    # psum/exp, t0+2 at 256 (q 256:512).  All mask multiplies
                    # are [128,128] against the shared lower-triangle master.
                    qt = qt_of[c]
                    t0 = 4 * g
                    # full tile t0
                    ps_a = ps.tile([P, CHUNK], F32, name="pss",
                                   tag=next_score_bank())
                    nc.tensor.matmul(ps_a[:], kslice(t0), qt[h][:],
                                     start=True, stop=True,
                                     skip_group_check=True)
                    ea = exp_pool.tile([P, CHUNK], BF16, name="exp", tag="exp")
                    nc.scalar.activation(ea[:], ps_a[:],
                                         mybir.ActivationFunctionType.Exp,
                                         scale=SCALE)
                    nc.vector.tensor_mul(ea[:, 0:P], ea[:, 0:P], master[:])
                    # packed tile: t0+1 at cols [0:384] (q 128:512),
                    # t0+3 at cols [384:512] (q 384:512)
                    ps_b = ps.tile([P, CHUNK], F32, name="pss",
                                   tag=next_score_bank())
                    nc.tensor.matmul(ps_b[:, 0:384], kslice(t0 + 1),
                                     qt[h][:, P:CHUNK],
                                     start=True, stop=True,
                                     skip_group_check=True)
                    nc.tensor.matmul(ps_b[:, 384:CHUNK], kslice(t0 + 3),
                                     qt[h][:, 384:CHUNK],
                                     start=False, stop=True,
                                     skip_group_check=True)
                    eb = exp_pool.tile([P, CHUNK], BF16, name="exp", tag="exp")
                    nc.scalar.activation(eb[:], ps_b[:],
                                         mybir.ActivationFunctionType.Exp,
                                         scale=SCALE)
                    nc.vector.tensor_mul(eb[:, 0:P], eb[:, 0:P], master[:])
                    nc.vector.tensor_mul(eb[:, 384:CHUNK], eb[:, 384:CHUNK],
                                         master[:])
                    # tile t0+2 at 256 (q 256:512)
                    ps_c = ps.tile([P, CHUNK], F32, name="pss",
                                   tag=next_score_bank())
                    nc.tensor.matmul(ps_c[:, 0:256], kslice(t0 + 2),
                                     qt[h][:, 256:CHUNK],
                                     start=True, stop=True,
                                     skip_group_check=True)
                    ec = exp_pool.tile([P, CHUNK], BF16, name="exp", tag="exp")
                    nc.scalar.activation(ec[:, 0:256], ps_c[:, 0:256],
                                         mybir.ActivationFunctionType.Exp,
                                         scale=SCALE)
                    nc.vector.tensor_mul(ec[:, 0:P], ec[:, 0:P], master[:])
                    state["ea"], state["eb"], state["ec"] = ea, eb, ec
                    # diagonal contributions to the running sum, with column
                    # realignment for the packed tiles
                    gst = state["gst"]
                    if g == 0:
                        nc.vector.tensor_copy(gst[:], ea[:])
                    else:
                        nc.vector.tensor_add(gst[:], gst[:], ea[:])
                    nc.vector.tensor_add(gst[:, P:CHUNK], gst[:, P:CHUNK],
                                         eb[:, 0:384])
                    nc.vector.tensor_add(gst[:, 256:CHUNK], gst[:, 256:CHUNK],
                                         ec[:, 0:256])
                    nc.vector.tensor_add(gst[:, 384:CHUNK], gst[:, 384:CHUNK],
                                         eb[:, 384:CHUNK])
                    if g >= 1:
                        for i in range(4):
                            emit_pv(4 * (g - 1) + i)

                def a_flush(g):
                    t0 = 4 * g
                    ea, eb, ec = state["ea"], state["eb"], state["ec"]
                    emit_pv(t0, ex=ea)
                    emit_pv(t0 + 1, qs=slice(P, CHUNK), ex=eb,
                            exs=slice(0, 384))
                    emit_pv(t0 + 2, qs=slice(256, CHUNK), ex=ec,
                            exs=slice(0, 256))
                    emit_pv(t0 + 3, qs=slice(384, CHUNK), ex=eb,
                            exs=slice(384, CHUNK), stop=True)

                def a_fin():
                    # single partition-reduce of the running exp sum; the
                    # all-ones stationary replicates the sum across all 128
                    # partitions, so no separate broadcast is needed.
                    rs = ps.tile([P, CHUNK], F32, name="rs",
                                 tag=next_score_bank())
                    nc.tensor.matmul(rs[:], ones[:], state["gst"][:],
                                     start=True, stop=True,
                                     skip_group_check=True)
                    zrec = rb_pool.tile([P, CHUNK], F32, name="zrec",
                                        tag="zr")
                    nc.vector.reciprocal_approx_fast(out=zrec[:], in_=rs[:])
                    ot = ot_pool.tile([P, CHUNK], BF16, name="ot", tag="ot")
                    nc.vector.tensor_mul(ot[:], state["oa"][:], zrec[:])
                    ots_of.setdefault(c, []).append(ot)

                units.append(a_start)
                for g in range(G - 1):
                    units.append(lambda g=g: a_group(g))
                units.append(lambda: a_group_diag(G - 1))
                units.append(lambda: a_flush(G - 1))
                units.append(a_fin)
                return units

            # ---------------- output projection ----------------
            def wo_units(c, banks=None, split_store=False):
                ssl = slice(c * CHUNK, (c + 1) * CHUNK)
                units = []

                def w_j2(j2):
                    ots = ots_of[c]
                    # banks=None -> share the score rotation (used when woven
                    # between attention groups, so psum allocation stays a
                    # single uniform round-robin at maximum depth)
                    tag = (next_score_bank() if banks is None
                           else banks[j2 % len(banks)])
                    pw = ps.tile([P, CHUNK], F32, name="pw", tag=tag)
                    for h in range(NH):
                        nc.tensor.matmul(
                            pw[:], woT_t[h][:, j2 * P:(j2 + 1) * P], ots[h][:],
                            start=(h == 0), stop=(h == NH - 1),
                            skip_group_check=True,
                        )
                    ob = osb_pool.tile([P, CHUNK], BF16, name="ob", tag="ob")
                    nc.vector.tensor_copy(ob[:], pw[:])
                    # the final chunk's stores alternate queues so the last
                    # output drain uses both DMA paths (scalar is exp-free by
                    # then); the last few split into halves across both
                    # queues to shrink the end-of-kernel transfer drain
                    if split_store and j2 >= DT - 4:
                        HC = CHUNK // 2
                        s0 = c * CHUNK
                        nc.sync.dma_start(
                            out=outT[j2 * P:(j2 + 1) * P, s0:s0 + HC],
                            in_=ob[:, 0:HC])
                        nc.scalar.dma_start(
                            out=outT[j2 * P:(j2 + 1) * P, s0 + HC:s0 + CHUNK],
                            in_=ob[:, HC:CHUNK])
                        return
                    eng = nc.scalar if (split_store and j2 % 2 == 1) else nc.sync
                    eng.dma_start(out=outT[j2 * P:(j2 + 1) * P, ssl],
                                  in_=ob[:], )

                for j2 in range(DT):
                    units.append(lambda j2=j2: w_j2(j2))
                return units

            def run(units):
                for u in units:
                    u()

            def weave(primary, filler):
                """Emit primary units with filler spread evenly between."""
                n, m = len(primary), len(filler)
                fi = 0
                for i, u in enumerate(primary):
                    u()
                    target = ((i + 1) * m) // n
                    while fi < target:
                        filler[fi]()
                        fi += 1
                while fi < m:
                    filler[fi]()
                    fi += 1

            # ---------------- software pipeline ----------------
            p0k, p0qv = proj0_units()
            run(p0k)
            weave(p0qv, xpre_units(1))
            for c in range(1, NCH):
                A = []
                for h in range(NH):
                    A.extend(attn_units(c - 1, h))
                # filler: x prefetch first, then the two-loops-back wo (its
                # casts spread between attention DVE ops instead of forming
                # a solid block), then this chunk's projections.
                F = []
                if c + 1 < NCH:
                    F.extend(xpre_units(c + 1))
                if c >= 2:
                    F.extend(wo_units(c - 2))
                F.extend(proj_units(c))
                weave(A, F)
            # tail: attn(3) woven with wo(2); a few wo(2) units are held back
            # and emitted AFTER the last head's finalize (so the finalize's
            # vector ops aren't queued behind their casts) but their matmuls
            # still feed the PE while the finalize chain completes.
            A = []
            for h in range(NH):
                A.extend(attn_units(NCH - 1, h))
            wo2 = wo_units(NCH - 2, banks=["b0", "b1"])
            weave(A[:-1], wo2[:14])
            run(A[-1:])
            run(wo2[14:])
            run(wo_units(NCH - 1, banks=["b0", "b1", "b2", "b3", "b4"],
                         split_store=True))

    nc.compile()
    return nc


_NC_CACHE = None


def _get_nc():
    global _NC_CACHE
    if _NC_CACHE is None:
        _NC_CACHE = build_kernel()
    return _NC_CACHE


def make_in_maps(x, wq, wk, wv, wo):
    bf = ml_dtypes.bfloat16
    in_maps = []
    for core in range(8):
        b, g = core // 4, core % 4
        j0 = g * JL
        in_maps.append({
            "xT": np.ascontiguousarray(x[b].T).astype(bf),
            "wqT": np.ascontiguousarray(wq[j0:j0 + JL, :].T).astype(bf),
            "wkT": np.ascontiguousarray(wk[j0:j0 + JL, :].T).astype(bf),
            "wvT": np.ascontiguousarray(wv[j0:j0 + JL, :].T).astype(bf),
            "woT": np.ascontiguousarray(wo[:, j0:j0 + JL].T).astype(bf),
        })
    return in_maps


def kernel(x, freqs_complex=None, mask=None, wq=None, wk=None, wv=None, wo=None,
           **_unused):
    x = np.asarray(x, dtype=np.float32)
    wq = np.asarray(wq, dtype=np.float32)
    wk = np.asarray(wk, dtype=np.float32)
    wv = np.asarray(wv, dtype=np.float32)
    wo = np.asarray(wo, dtype=np.float32)

    nc = _get_nc()
    in_maps = make_in_maps(x, wq, wk, wv, wo)
    res = run_bass_kernel_spmd(nc, in_maps, list(range(8)))

    out = np.zeros((B, S, D), dtype=np.float32)
    for core in range(8):
        out[core // 4] += res.results[core]["outT"].T.astype(np.float32)
    return out
